# revision 49
# baseline (speedup 1.0000x reference)
"""Causal self-attention (B=2, S=2048, E=1024, H=16) on 8 TRN2 NeuronCores.

Sharding: core c = 4*b + g handles batch b and head-group g (4 heads,
256 E-columns). Each core computes q/k/v projections for its head slice,
causal attention for its 4 heads, and a partial output projection
y_c = ctx_g @ Wo[rows_g]. Host sums the 4 partials per batch and adds bo.

Engine plan (CoreSim cost model driven; ~100us/core vs 172.6us baseline):
  PE   - QKV projections in fp8e4 DoubleRow (K=256/instr, 0.5 cyc/row) with
         3-term error compensation (x8@W8 + x8@rW8 + rx8@W8) accumulated in
         one PSUM group; scores in fp8 DoubleRow with the head dim folded
         32x2 onto partitions (q/k re-quantized to fp8; half-rate scores);
         A@V in natural orientation (out = [128 q-parts, 65] bf16, the
         denominator rides as V's ones column; all 4 heads of a stile share
         one PSUM bank via per-byte zero-region semantics + a nosync order
         edge on the single start=True matmul); out-proj in bf16 from a
         folded ctxT.
  ACT  - exp only (the floor: ~8.4M exps/core = 72.8us; the emission order
         makes exp the continuously-fed pacer; chunks processed 0,1,3,2 so
         the smallest remaining tail follows the last exp).
  DVE  - q/k bias+descale (psum->fp8), v bias+descale, denominators'
         reciprocal + ctx normalize, y psum->sbuf (GPSIMD cannot touch
         PSUM on real HW - walrus birverifier enforces this).
  Pool - causal mask multiply on te (sbuf only), small-constant DMAs.
  DMA  - fp8/bf16 inputs, q/k fp8 fold via partition-shift SBUF DMAs,
         XBAR dma_start_transpose builds ctxT off-engine, bf16 y partials.

Emission = Tile scheduler priority: [all scores/exp chains, chunk-ordered,
with the next wave's q/k proj between chunks] then [v proj + ctx + out-proj
as hole-fillers] - deferrable PE work migrates into exp-paced windows.

PSUM banks (8): scores ring [128,2,512]x2 = 4 (bank-per-head-pair),
ctx/v ring [128,512]x2 = 2 (4 heads per bank), qk-proj/out-proj ring
[128,512]x2 = 2.
"""

import os

import numpy as np
import ml_dtypes

os.environ.setdefault("NEURON_RT_RESET_CORES", "1")

B, S, E, H, D = 2, 2048, 1024, 16, 64
NCORES = 8
EC = 256          # E-columns per core (4 heads x 64)
QC = 512          # q-chunk width
NQC = S // QC     # 4
NKT = S // 128    # 16 k-tiles
NKS = 4           # fp8 DoubleRow K-steps (4 x 256 = 1024)

WSQ = 256.0       # fp8 range scale for Wq*(1/8)
WSK = 64.0        # fp8 range scale for Wk / Wv
E4NP = ml_dtypes.float8_e4m3
BFNP = ml_dtypes.bfloat16

_CACHE = {}


def _build_nc(cfg=None):
    cfg = cfg or {}
    import concourse.mybir as mybir
    import concourse.tile as tile
    import concourse.bass as bass
    from concourse import bacc

    F32 = mybir.dt.float32
    BF16 = mybir.dt.bfloat16
    F8 = mybir.dt.float8e4
    EXP = mybir.ActivationFunctionType.Exp
    DR = mybir.MatmulPerfMode.DoubleRow
    MUL = mybir.AluOpType.mult
    ADD = mybir.AluOpType.add

    TE_BUFS = cfg.get("te", 44)
    nc = bacc.Bacc("TRN2", target_bir_lowering=False, debug=False)

    x8 = nc.dram_tensor("x8", [128, NKS, 2, S], F8, kind="ExternalInput")
    rx8 = nc.dram_tensor("rx8", [128, NKS, 2, S], F8, kind="ExternalInput")
    wq8 = nc.dram_tensor("wq8", [128, NKS, 2, EC], F8, kind="ExternalInput")
    wk8 = nc.dram_tensor("wk8", [128, NKS, 2, EC], F8, kind="ExternalInput")
    wv8 = nc.dram_tensor("wv8", [128, NKS, 2, EC], F8, kind="ExternalInput")
    rwq8 = nc.dram_tensor("rwq8", [128, NKS, 2, EC], F8, kind="ExternalInput")
    rwk8 = nc.dram_tensor("rwk8", [128, NKS, 2, EC], F8, kind="ExternalInput")
    rwv8 = nc.dram_tensor("rwv8", [128, NKS, 2, EC], F8, kind="ExternalInput")
    wo = nc.dram_tensor("wo", [128, 2, E], BF16, kind="ExternalInput")
    bq = nc.dram_tensor("bq", [2, 128, 1], F32, kind="ExternalInput")
    bk = nc.dram_tensor("bk", [2, 128, 1], F32, kind="ExternalInput")
    bv = nc.dram_tensor("bv", [1, EC], F32, kind="ExternalInput")
    msk = nc.dram_tensor("msk", [128, 128], BF16, kind="ExternalInput")
    ones = nc.dram_tensor("ones", [1, 64], BF16, kind="ExternalInput")

    y = nc.dram_tensor("y", [S, E], BF16, kind="ExternalOutput")

    with tile.TileContext(nc) as tc:
        with (
            tc.tile_pool(name="weights", bufs=1) as wpool,
            tc.tile_pool(name="xp", bufs=1) as xp,
            tc.tile_pool(name="qkv", bufs=1) as qkv,
            tc.tile_pool(name="tep", bufs=TE_BUFS) as tep,
            tc.tile_pool(name="tcnp", bufs=4) as tcnp,
            tc.tile_pool(name="rcp", bufs=4) as rcp,
            tc.tile_pool(name="ctp", bufs=1) as ctp,
            tc.tile_pool(name="typ", bufs=6) as typ,
            tc.tile_pool(name="smalls", bufs=1) as smalls,
            tc.tile_pool(name="scp", bufs=2, space="PSUM") as scp,
            tc.tile_pool(name="cxp", bufs=2, space="PSUM") as cxp,
            tc.tile_pool(name="prp", bufs=2, space="PSUM") as prp,
        ):
            # ---- small constants (Pool SWDGE queue) ----
            tbq = smalls.tile([128, 2], F32, tag="bq")
            tbk = smalls.tile([128, 2], F32, tag="bk")
            tbv = smalls.tile([128, EC], F32, tag="bv")
            tmsk = smalls.tile([128, 128], BF16, tag="msk")

            for r in range(2):
                nc.gpsimd.dma_start(tbq[:, r:r + 1], bq[r])
                nc.gpsimd.dma_start(tbk[:, r:r + 1], bk[r])
            bvap = bv[0, :]
            bv_b = bass.AP(tensor=bvap.tensor, offset=bvap.offset,
                           ap=[[0, 128]] + list(bvap.ap))
            nc.gpsimd.dma_start(tbv[:], bv_b)
            nc.gpsimd.dma_start(tmsk[:], msk[:])

            # ---- x fp8 (+ residual), loaded per 512-wide wave ----
            tx = xp.tile([128, NKS, 2, S], F8, tag="x8")
            trx = xp.tile([128, NKS, 2, S], F8, tag="rx8")

            def ldx(w):
                sl = slice(w * QC, (w + 1) * QC)
                nc.scalar.dma_start(tx[:, :, :, sl], x8[:, :, :, sl])
                nc.scalar.dma_start(trx[:, :, :, sl], rx8[:, :, :, sl])

            # wave-0 x + q/k weights first so the q/k chain starts ASAP
            sl0 = slice(0, QC)
            tw = {}

            def ldw(nm, dram):
                t = wpool.tile([128, NKS, 2, EC], F8, tag=nm, name=nm)
                nc.sync.dma_start(t[:], dram[:])
                tw[nm] = t

            ldw("wq", wq8)
            ldw("wk", wk8)
            for ks in range(NKS):
                nc.scalar.dma_start(tx[:, ks, :, sl0], x8[:, ks, :, sl0])
            ldw("rwq", rwq8)
            ldw("rwk", rwk8)
            nc.scalar.dma_start(trx[:, :, :, sl0], rx8[:, :, :, sl0])
            ldw("wv", wv8)
            ldw("rwv", rwv8)
            two = wpool.tile([128, 2, E], BF16, tag="wo")
            nc.sync.dma_start(two[:], wo[:])
            for w_ in range(1, NQC):
                ldx(w_)

            # ---- persistent activations ----
            SCF8 = cfg.get("sc_fp8", True)
            QKDT = F8 if SCF8 else BF16
            # natural layout (partition = feature within r-tile)
            tq = [qkv.tile([128, S], QKDT, tag=f"q{r}", name=f"q{r}")
                  for r in range(2)]
            tk = [qkv.tile([128, S], QKDT, tag=f"k{r}", name=f"k{r}")
                  for r in range(2)]
            # folded fp8 layout: partition = head_local*32 + d_low,
            # slot = d_high half; built by partition-shift DMAs
            if SCF8:
                tqf = qkv.tile([128, 2, S], F8, tag="qf", name="qf")
                tkf = qkv.tile([128, 2, S], F8, tag="kf", name="kf")
            # v1: [128, kt, head, 65]; col 64 of each head block = 1.0
            tv = qkv.tile([128, NKT, 4, 65], BF16, tag="v")
            onesap = ones[0, 0:1]
            ones_v = bass.AP(tensor=onesap.tensor, offset=onesap.offset,
                             ap=[[0, 128], [0, NKT * 4], [0, 1]])
            nc.gpsimd.dma_start(tv[:, :, :, 64:65], ones_v)

            tct = ctp.tile([128, 2, S], BF16, tag="ct")  # folded ctxT

            def mask_b(n):
                m = tmsk[:]
                return bass.AP(tensor=m.tensor, offset=m.offset,
                               ap=[list(m.ap[0]), [0, 2], [1, n]])

            def colb(ap1, n):
                # [128, 1] AP -> [128, n] stride-0 broadcast
                return bass.AP(tensor=ap1.tensor, offset=ap1.offset,
                               ap=[list(ap1.ap[0]), [0, n]])

            def colb2(ap2, n):
                # [128, k] AP -> [128, k, n] stride-0 broadcast
                return bass.AP(tensor=ap2.tensor, offset=ap2.offset,
                               ap=[list(ap2.ap[0]), list(ap2.ap[1]), [0, n]])

            def apx(t, off, dims):
                # partition dim of tile t + custom free dims at f32-col offset
                a = t[:]
                return bass.AP(tensor=a.tensor, offset=a.offset + off,
                               ap=[list(a.ap[0])] + [list(d) for d in dims])

            def order(later, first):
                bass._add_dep_helper(later.ins, first.ins, sync=False,
                                     reason="psum zero-region order")

            COMP = ((None, tx), ("r", tx), (None, trx))  # (w-residual?, x-tensor)

            def qk_unit(w, r, wn):
                dst, ws, bias = ((tq, WSQ, tbq) if wn == "wq"
                                 else (tk, WSK, tbk))
                sl = slice(w * QC, (w + 1) * QC)
                prt = prp.tile([128, QC], F32, tag="pr", name=f"p{wn}{w}_{r}")
                idx = 0
                for res, xt in COMP:
                    wt = tw[("r" if res else "") + wn]
                    for ks in range(NKS):
                        nc.tensor.matmul(
                            prt[:],
                            wt[:, ks, :, r * 128:(r + 1) * 128],
                            xt[:, ks, :, sl],
                            start=(idx == 0), stop=(idx == 3 * NKS - 1),
                            perf_mode=DR)
                        idx += 1
                nc.vector.scalar_tensor_tensor(
                    dst[r][:, sl], prt[:], 1.0 / ws,
                    colb(bias[:, r:r + 1], QC), op0=MUL, op1=ADD)
                if SCF8:
                    fdst = tqf if wn == "wq" else tkf
                    for h2 in range(2):
                        for slot in range(2):
                            nc.sync.dma_start(
                                fdst[(2 * r + h2) * 32:(2 * r + h2) * 32 + 32,
                                     slot, sl],
                                dst[r][h2 * 64 + slot * 32:
                                       h2 * 64 + slot * 32 + 32, sl])

            def v_unit(w, pair):
                cxt = cxp.tile([128, 512], F32, tag="cx", name=f"pv{w}_{pair}")
                first = None
                for st2 in range(2):
                    st = 4 * w + 2 * pair + st2
                    idx = 0
                    for res, xt in COMP:
                        wt = tw[("r" if res else "") + "wv"]
                        for ks in range(NKS):
                            m = nc.tensor.matmul(
                                cxt[:, st2 * EC:(st2 + 1) * EC],
                                xt[:, ks, :, st * 128:(st + 1) * 128],
                                wt[:, ks, :, :],
                                start=(first is None), stop=False,
                                perf_mode=DR, skip_group_check=True)
                            if first is None:
                                first = m
                            elif idx == 0:
                                order(m, first)
                            idx += 1
                for st2 in range(2):
                    st = 4 * w + 2 * pair + st2
                    nc.vector.scalar_tensor_tensor(
                        tv[:, st, :, 0:64],
                        apx(cxt, st2 * EC, [[64, 4], [1, 64]]),
                        1.0 / WSK,
                        tbv[:].rearrange("p (h d) -> p h d", h=4),
                        op0=MUL, op1=ADD)

            def proj_wave(w):
                for r in range(2):
                    qk_unit(w, r, "wq")
                    qk_unit(w, r, "wk")
                for p in range(2):
                    v_unit(w, p)

            all_tes = {}

            def scores_phase(qc):
                n_kt = 4 * (qc + 1)
                tes = [[], []]
                all_tes[qc] = tes
                for hp in range(2):
                    for kt in range(n_kt):
                        dg = kt - 4 * qc
                        coff = 128 * dg if dg > 0 else 0
                        sct = scp.tile([128, 2, QC], F32, tag="sc",
                                       name=f"s{qc}_{hp}_{kt}")
                        for h2 in range(2):
                            if SCF8:
                                hl = (2 * hp + h2) * 32
                                nc.tensor.matmul(
                                    sct[:, h2, coff:QC],
                                    tkf[hl:hl + 32, :,
                                        kt * 128:(kt + 1) * 128],
                                    tqf[hl:hl + 32, :,
                                        qc * QC + coff:(qc + 1) * QC],
                                    start=True, stop=True, perf_mode=DR,
                                    tile_position=(hl, 0))
                            else:
                                bp = h2 * 64
                                nc.tensor.matmul(
                                    sct[:, h2, coff:QC],
                                    tk[hp][bp:bp + 64, kt * 128:(kt + 1) * 128],
                                    tq[hp][bp:bp + 64,
                                           qc * QC + coff:(qc + 1) * QC],
                                    start=True, stop=True)
                        tet = tep.tile([128, 2, QC], BF16, tag="te",
                                       name=f"te{qc}_{hp}_{kt}")
                        nc.scalar.activation(
                            tet[:, :, coff:QC], sct[:, :, coff:QC], EXP)
                        if dg >= 0:
                            nc.gpsimd.tensor_mul(
                                tet[:, :, coff:coff + 128],
                                tet[:, :, coff:coff + 128],
                                mask_b(128))
                        tes[hp].append(tet)

            def ctx_phase(qc, op_inline=False):
                tes = all_tes[qc]
                for sti in range(4):
                    st = 4 * qc + sti
                    cxt = cxp.tile([128, 512], F32, tag="cx",
                                   name=f"cx{qc}_{sti}")
                    first = None
                    for hp in range(2):
                        for h2 in range(2):
                            h = 2 * hp + h2
                            for kt in range(st + 1):
                                m = nc.tensor.matmul(
                                    cxt[:, h * 128:h * 128 + 65],
                                    tes[hp][kt][:, h2,
                                                sti * 128:(sti + 1) * 128],
                                    tv[:, kt, h, :],
                                    start=(first is None), stop=False,
                                    skip_group_check=True)
                                if first is None:
                                    first = m
                                elif kt == 0:
                                    order(m, first)
                    rc = rcp.tile([128, 4], F32, tag="rc",
                                  name=f"rc{qc}_{sti}")
                    nc.vector.reciprocal(rc[:], apx(cxt, 64, [[128, 4], [1, 1]]))
                    tcn = tcnp.tile([128, 4, 64], BF16, tag="cn",
                                    name=f"cn{qc}_{sti}")
                    nc.vector.tensor_mul(
                        tcn[:], apx(cxt, 0, [[128, 4], [1, 64]]),
                        colb2(rc[:], 64))
                    for hp in range(2):
                        nc.sync.dma_start_transpose(
                            tct[:, hp, st * 128:(st + 1) * 128],
                            tcn[:, 2 * hp:2 * hp + 2, :])
                    if op_inline:
                        out_proj_sti(qc, sti)

            def out_proj_sti(qc, sti):
                    st = 4 * qc + sti
                    for nn in range(2):
                        pyt = prp.tile([128, QC], F32, tag="pr",
                                       name=f"py{qc}_{sti}_{nn}")
                        for hp in range(2):
                            nc.tensor.matmul(
                                pyt[:],
                                tct[:, hp, st * 128:(st + 1) * 128],
                                two[:, hp, nn * QC:(nn + 1) * QC],
                                start=(hp == 0), stop=(hp == 1))
                        ty = typ.tile([128, QC], BF16, tag="y",
                                      name=f"y{qc}_{sti}_{nn}")
                        nc.vector.tensor_copy(ty[:], pyt[:])
                        nc.sync.dma_start(
                            y[st * 128:(st + 1) * 128,
                              nn * QC:(nn + 1) * QC], ty[:])

            def out_proj(qc):
                for sti in range(4):
                    out_proj_sti(qc, sti)

            # Emission = scheduler priority. The exp stream is the global
            # pacer: scores/exp chains first (chunk order) with the next
            # wave's q/k projection between them, then all deferrable work.
            CORDER = cfg.get("corder", [0, 1, 3, 2])
            # waves needed before a chunk's scores: all w <= qc
            emitted_qk = set()

            def need_qk(qc):
                for w_ in range(qc + 1):
                    if w_ not in emitted_qk:
                        emitted_qk.add(w_)
                        for r in range(2):
                            qk_unit(w_, r, "wq")
                            qk_unit(w_, r, "wk")

            need_qk(CORDER[0])
            for i, qc in enumerate(CORDER):
                scores_phase(qc)
                if i + 1 < NQC:
                    need_qk(CORDER[i + 1])
            emitted_v = set()

            def need_v(qc):
                for w_ in range(qc + 1):
                    if w_ not in emitted_v:
                        emitted_v.add(w_)
                        for p in range(2):
                            v_unit(w_, p)

            need_v(CORDER[0])
            for i, qc in enumerate(CORDER):
                if i + 1 < NQC:
                    need_v(CORDER[i + 1])
                ctx_phase(qc)
                out_proj(qc)

    nc.compile()
    return nc


def _get_nc():
    if "nc" not in _CACHE:
        _CACHE["nc"] = _build_nc()
    return _CACHE["nc"]


def make_mask():
    kl = np.arange(128)[:, None]
    ql = np.arange(128)[None, :]
    return (ql >= kl).astype(BFNP)


def _fold(t):
    # [E, N] -> [128, NKS, 2, N] with e = ks*256 + sl*128 + p
    n = t.shape[1]
    return np.ascontiguousarray(
        t.reshape(NKS, 2, 128, n).transpose(2, 0, 1, 3))


def _q8(t):
    t8 = t.astype(E4NP)
    return t8, (t - t8.astype(np.float32)).astype(E4NP)


def shard_inputs(x, Wq, bq, Wk, bk, Wv, bv, Wo, bo):
    x = np.asarray(x, dtype=np.float32)
    scale = np.float32(1.0 / np.sqrt(D))
    mask = make_mask()
    ones = np.ones((1, 64), BFNP)
    in_maps = []
    xf = []
    for b in range(B):
        x8, rx8 = _q8(_fold(np.ascontiguousarray(x[b].T)))
        xf.append((x8, rx8))
    for c in range(NCORES):
        b, g = divmod(c, 4)
        cs = slice(g * EC, (g + 1) * EC)
        wq, rwq = _q8(_fold(np.asarray(Wq[:, cs]) * (scale * WSQ)))
        wk, rwk = _q8(_fold(np.asarray(Wk[:, cs]) * WSK))
        wv, rwv = _q8(_fold(np.asarray(Wv[:, cs]) * WSK))
        in_maps.append({
            "x8": xf[b][0], "rx8": xf[b][1],
            "wq8": wq, "rwq8": rwq,
            "wk8": wk, "rwk8": rwk,
            "wv8": wv, "rwv8": rwv,
            "wo": np.ascontiguousarray(
                np.asarray(Wo[cs, :]).reshape(2, 128, E).transpose(1, 0, 2)
            ).astype(BFNP),
            "bq": (np.asarray(bq[cs]) * scale).reshape(2, 128, 1).astype(np.float32),
            "bk": np.asarray(bk[cs]).reshape(2, 128, 1).astype(np.float32),
            "bv": np.asarray(bv[cs]).reshape(1, EC).astype(np.float32),
            "msk": mask,
            "ones": ones,
        })
    return in_maps


def combine_outputs(results, bo):
    y = np.zeros((B, S, E), np.float32)
    for c in range(NCORES):
        b = c // 4
        y[b] += np.asarray(results[c]["y"], dtype=np.float32)
    y += np.asarray(bo, dtype=np.float32)[None, None, :]
    return y


def kernel(x, Wq, bq, Wk, bk, Wv, bv, Wo, bo):
    from concourse.bass_utils import run_bass_kernel_spmd

    nc = _get_nc()
    in_maps = shard_inputs(x, Wq, bq, Wk, bk, Wv, bv, Wo, bo)
    try:
        res = run_bass_kernel_spmd(nc, in_maps, core_ids=list(range(NCORES)))
    except Exception:
        # transient device errors (e.g. a wedged core) usually clear on retry
        res = run_bass_kernel_spmd(nc, in_maps, core_ids=list(range(NCORES)))
    return combine_outputs(res.results, bo)


# revision 54
# speedup vs baseline: 1.0039x; 1.0039x over previous
"""Causal self-attention (B=2, S=2048, E=1024, H=16) on 8 TRN2 NeuronCores.

Sharding: core c = 4*b + g handles batch b and head-group g (4 heads,
256 E-columns). Each core computes q/k/v projections for its head slice,
causal attention for its 4 heads, and a partial output projection
y_c = ctx_g @ Wo[rows_g]. Host sums the 4 partials per batch and adds bo.

Engine plan (CoreSim cost model driven; ~100us/core vs 172.6us baseline):
  PE   - QKV projections in fp8e4 DoubleRow (K=256/instr, 0.5 cyc/row) with
         3-term error compensation (x8@W8 + x8@rW8 + rx8@W8) accumulated in
         one PSUM group; scores in fp8 DoubleRow with the head dim folded
         32x2 onto partitions (q/k re-quantized to fp8; half-rate scores);
         A@V in natural orientation (out = [128 q-parts, 65] bf16, the
         denominator rides as V's ones column; all 4 heads of a stile share
         one PSUM bank via per-byte zero-region semantics + a nosync order
         edge on the single start=True matmul); out-proj in bf16 from a
         folded ctxT.
  ACT  - exp only (the floor: ~8.4M exps/core = 72.8us; the emission order
         makes exp the continuously-fed pacer; chunks processed 0,1,3,2 so
         the smallest remaining tail follows the last exp).
  DVE  - q/k bias+descale (psum->fp8), v bias+descale, denominators'
         reciprocal + ctx normalize, y psum->sbuf (GPSIMD cannot touch
         PSUM on real HW - walrus birverifier enforces this).
  Pool - causal mask multiply on te (sbuf only), small-constant DMAs.
  DMA  - fp8/bf16 inputs, q/k fp8 fold via partition-shift SBUF DMAs,
         XBAR dma_start_transpose builds ctxT off-engine, bf16 y partials.

Emission = Tile scheduler priority: [all scores/exp chains, chunk-ordered,
with the next wave's q/k proj between chunks] then [v proj + ctx + out-proj
as hole-fillers] - deferrable PE work migrates into exp-paced windows.

PSUM banks (8): scores ring [128,2,512]x2 = 4 (bank-per-head-pair),
ctx/v ring [128,512]x2 = 2 (4 heads per bank), qk-proj/out-proj ring
[128,512]x2 = 2.
"""

import os

import numpy as np
import ml_dtypes

os.environ.setdefault("NEURON_RT_RESET_CORES", "1")

B, S, E, H, D = 2, 2048, 1024, 16, 64
NCORES = 8
EC = 256          # E-columns per core (4 heads x 64)
QC = 512          # q-chunk width
NQC = S // QC     # 4
NKT = S // 128    # 16 k-tiles
NKS = 4           # fp8 DoubleRow K-steps (4 x 256 = 1024)

WSQ = 256.0       # fp8 range scale for Wq*(1/8)
WSK = 64.0        # fp8 range scale for Wk / Wv
E4NP = ml_dtypes.float8_e4m3
BFNP = ml_dtypes.bfloat16

_CACHE = {}


def _build_nc(cfg=None):
    cfg = cfg or {}
    import concourse.mybir as mybir
    import concourse.tile as tile
    import concourse.bass as bass
    from concourse import bacc

    F32 = mybir.dt.float32
    BF16 = mybir.dt.bfloat16
    F8 = mybir.dt.float8e4
    EXP = mybir.ActivationFunctionType.Exp
    DR = mybir.MatmulPerfMode.DoubleRow
    MUL = mybir.AluOpType.mult
    ADD = mybir.AluOpType.add

    TE_BUFS = cfg.get("te", 44)
    nc = bacc.Bacc("TRN2", target_bir_lowering=False, debug=False)

    x8 = nc.dram_tensor("x8", [128, NKS, 2, S], F8, kind="ExternalInput")
    rx8 = nc.dram_tensor("rx8", [128, NKS, 2, S], F8, kind="ExternalInput")
    wq8 = nc.dram_tensor("wq8", [128, NKS, 2, EC], F8, kind="ExternalInput")
    wk8 = nc.dram_tensor("wk8", [128, NKS, 2, EC], F8, kind="ExternalInput")
    wv8 = nc.dram_tensor("wv8", [128, NKS, 2, EC], F8, kind="ExternalInput")
    rwq8 = nc.dram_tensor("rwq8", [128, NKS, 2, EC], F8, kind="ExternalInput")
    rwk8 = nc.dram_tensor("rwk8", [128, NKS, 2, EC], F8, kind="ExternalInput")
    rwv8 = nc.dram_tensor("rwv8", [128, NKS, 2, EC], F8, kind="ExternalInput")
    wo = nc.dram_tensor("wo", [128, 2, E], BF16, kind="ExternalInput")
    bq = nc.dram_tensor("bq", [2, 128, 1], F32, kind="ExternalInput")
    bk = nc.dram_tensor("bk", [2, 128, 1], F32, kind="ExternalInput")
    bv = nc.dram_tensor("bv", [1, EC], F32, kind="ExternalInput")
    msk = nc.dram_tensor("msk", [128, 128], BF16, kind="ExternalInput")
    ones = nc.dram_tensor("ones", [1, 64], BF16, kind="ExternalInput")

    y = nc.dram_tensor("y", [S, E], BF16, kind="ExternalOutput")

    with tile.TileContext(nc) as tc:
        with (
            tc.tile_pool(name="weights", bufs=1) as wpool,
            tc.tile_pool(name="xp", bufs=1) as xp,
            tc.tile_pool(name="qkv", bufs=1) as qkv,
            tc.tile_pool(name="tep", bufs=TE_BUFS) as tep,
            tc.tile_pool(name="tcnp", bufs=8) as tcnp,
            tc.tile_pool(name="rcp", bufs=4) as rcp,
            tc.tile_pool(name="ctp", bufs=1) as ctp,
            tc.tile_pool(name="typ", bufs=10) as typ,
            tc.tile_pool(name="smalls", bufs=1) as smalls,
            tc.tile_pool(name="scp", bufs=2, space="PSUM") as scp,
            tc.tile_pool(name="cxp", bufs=2, space="PSUM") as cxp,
            tc.tile_pool(name="prp", bufs=2, space="PSUM") as prp,
        ):
            # ---- small constants (Pool SWDGE queue) ----
            tbq = smalls.tile([128, 2], F32, tag="bq")
            tbk = smalls.tile([128, 2], F32, tag="bk")
            tbv = smalls.tile([128, EC], F32, tag="bv")
            tmsk = smalls.tile([128, 128], BF16, tag="msk")

            for r in range(2):
                nc.gpsimd.dma_start(tbq[:, r:r + 1], bq[r])
                nc.gpsimd.dma_start(tbk[:, r:r + 1], bk[r])
            bvap = bv[0, :]
            bv_b = bass.AP(tensor=bvap.tensor, offset=bvap.offset,
                           ap=[[0, 128]] + list(bvap.ap))
            nc.gpsimd.dma_start(tbv[:], bv_b)
            nc.gpsimd.dma_start(tmsk[:], msk[:])

            # ---- x fp8 (+ residual), loaded per 512-wide wave ----
            tx = xp.tile([128, NKS, 2, S], F8, tag="x8")
            trx = xp.tile([128, NKS, 2, S], F8, tag="rx8")

            def ldx(w):
                sl = slice(w * QC, (w + 1) * QC)
                nc.gpsimd.dma_start(tx[:, :, :, sl], x8[:, :, :, sl])
                nc.gpsimd.dma_start(trx[:, :, :, sl], rx8[:, :, :, sl])

            # wave-0 x + q/k weights first so the q/k chain starts ASAP
            sl0 = slice(0, QC)
            tw = {}

            def ldw(nm, dram):
                t = wpool.tile([128, NKS, 2, EC], F8, tag=nm, name=nm)
                nc.sync.dma_start(t[:], dram[:])
                tw[nm] = t

            ldw("wq", wq8)
            ldw("wk", wk8)
            for ks in range(NKS):
                nc.scalar.dma_start(tx[:, ks, :, sl0], x8[:, ks, :, sl0])
            ldw("rwq", rwq8)
            ldw("rwk", rwk8)
            nc.scalar.dma_start(trx[:, :, :, sl0], rx8[:, :, :, sl0])
            ldw("wv", wv8)
            ldw("rwv", rwv8)
            two = wpool.tile([128, 2, E], BF16, tag="wo")
            nc.sync.dma_start(two[:], wo[:])
            for w_ in range(1, NQC):
                ldx(w_)

            # ---- persistent activations ----
            SCF8 = cfg.get("sc_fp8", True)
            QKDT = F8 if SCF8 else BF16
            # natural layout (partition = feature within r-tile)
            tq = [qkv.tile([128, S], QKDT, tag=f"q{r}", name=f"q{r}")
                  for r in range(2)]
            tk = [qkv.tile([128, S], QKDT, tag=f"k{r}", name=f"k{r}")
                  for r in range(2)]
            # folded fp8 layout: partition = head_local*32 + d_low,
            # slot = d_high half; built by partition-shift DMAs
            if SCF8:
                tqf = qkv.tile([128, 2, S], F8, tag="qf", name="qf")
                tkf = qkv.tile([128, 2, S], F8, tag="kf", name="kf")
            # v1: [128, kt, head, 65]; col 64 of each head block = 1.0
            tv = qkv.tile([128, NKT, 4, 65], BF16, tag="v")
            onesap = ones[0, 0:1]
            ones_v = bass.AP(tensor=onesap.tensor, offset=onesap.offset,
                             ap=[[0, 128], [0, NKT * 4], [0, 1]])
            nc.gpsimd.dma_start(tv[:, :, :, 64:65], ones_v)

            tct = ctp.tile([128, 2, S], BF16, tag="ct")  # folded ctxT

            def mask_b(n):
                m = tmsk[:]
                return bass.AP(tensor=m.tensor, offset=m.offset,
                               ap=[list(m.ap[0]), [0, 2], [1, n]])

            def colb(ap1, n):
                # [128, 1] AP -> [128, n] stride-0 broadcast
                return bass.AP(tensor=ap1.tensor, offset=ap1.offset,
                               ap=[list(ap1.ap[0]), [0, n]])

            def colb2(ap2, n):
                # [128, k] AP -> [128, k, n] stride-0 broadcast
                return bass.AP(tensor=ap2.tensor, offset=ap2.offset,
                               ap=[list(ap2.ap[0]), list(ap2.ap[1]), [0, n]])

            def apx(t, off, dims):
                # partition dim of tile t + custom free dims at f32-col offset
                a = t[:]
                return bass.AP(tensor=a.tensor, offset=a.offset + off,
                               ap=[list(a.ap[0])] + [list(d) for d in dims])

            def order(later, first):
                bass._add_dep_helper(later.ins, first.ins, sync=False,
                                     reason="psum zero-region order")

            COMP = ((None, tx), ("r", tx), (None, trx))  # (w-residual?, x-tensor)

            def qk_unit(w, r, wn):
                dst, ws, bias = ((tq, WSQ, tbq) if wn == "wq"
                                 else (tk, WSK, tbk))
                sl = slice(w * QC, (w + 1) * QC)
                prt = prp.tile([128, QC], F32, tag="pr", name=f"p{wn}{w}_{r}")
                idx = 0
                for res, xt in COMP:
                    wt = tw[("r" if res else "") + wn]
                    for ks in range(NKS):
                        nc.tensor.matmul(
                            prt[:],
                            wt[:, ks, :, r * 128:(r + 1) * 128],
                            xt[:, ks, :, sl],
                            start=(idx == 0), stop=(idx == 3 * NKS - 1),
                            perf_mode=DR)
                        idx += 1
                nc.vector.scalar_tensor_tensor(
                    dst[r][:, sl], prt[:], 1.0 / ws,
                    colb(bias[:, r:r + 1], QC), op0=MUL, op1=ADD)
                if SCF8:
                    fdst = tqf if wn == "wq" else tkf
                    for h2 in range(2):
                        for slot in range(2):
                            nc.sync.dma_start(
                                fdst[(2 * r + h2) * 32:(2 * r + h2) * 32 + 32,
                                     slot, sl],
                                dst[r][h2 * 64 + slot * 32:
                                       h2 * 64 + slot * 32 + 32, sl])

            def v_unit(w, pair):
                cxt = cxp.tile([128, 512], F32, tag="cx", name=f"pv{w}_{pair}")
                first = None
                for st2 in range(2):
                    st = 4 * w + 2 * pair + st2
                    idx = 0
                    for res, xt in COMP:
                        wt = tw[("r" if res else "") + "wv"]
                        for ks in range(NKS):
                            m = nc.tensor.matmul(
                                cxt[:, st2 * EC:(st2 + 1) * EC],
                                xt[:, ks, :, st * 128:(st + 1) * 128],
                                wt[:, ks, :, :],
                                start=(first is None), stop=False,
                                perf_mode=DR, skip_group_check=True)
                            if first is None:
                                first = m
                            elif idx == 0:
                                order(m, first)
                            idx += 1
                for st2 in range(2):
                    st = 4 * w + 2 * pair + st2
                    nc.vector.scalar_tensor_tensor(
                        tv[:, st, :, 0:64],
                        apx(cxt, st2 * EC, [[64, 4], [1, 64]]),
                        1.0 / WSK,
                        tbv[:].rearrange("p (h d) -> p h d", h=4),
                        op0=MUL, op1=ADD)

            def proj_wave(w):
                for r in range(2):
                    qk_unit(w, r, "wq")
                    qk_unit(w, r, "wk")
                for p in range(2):
                    v_unit(w, p)

            all_tes = {}

            def scores_phase(qc):
                n_kt = 4 * (qc + 1)
                tes = [[], []]
                all_tes[qc] = tes
                for hp in range(2):
                    for kt in range(n_kt):
                        dg = kt - 4 * qc
                        coff = 128 * dg if dg > 0 else 0
                        sct = scp.tile([128, 2, QC], F32, tag="sc",
                                       name=f"s{qc}_{hp}_{kt}")
                        for h2 in range(2):
                            if SCF8:
                                hl = (2 * hp + h2) * 32
                                nc.tensor.matmul(
                                    sct[:, h2, coff:QC],
                                    tkf[hl:hl + 32, :,
                                        kt * 128:(kt + 1) * 128],
                                    tqf[hl:hl + 32, :,
                                        qc * QC + coff:(qc + 1) * QC],
                                    start=True, stop=True, perf_mode=DR,
                                    tile_position=(hl, 0))
                            else:
                                bp = h2 * 64
                                nc.tensor.matmul(
                                    sct[:, h2, coff:QC],
                                    tk[hp][bp:bp + 64, kt * 128:(kt + 1) * 128],
                                    tq[hp][bp:bp + 64,
                                           qc * QC + coff:(qc + 1) * QC],
                                    start=True, stop=True)
                        tet = tep.tile([128, 2, QC], BF16, tag="te",
                                       name=f"te{qc}_{hp}_{kt}")
                        nc.scalar.activation(
                            tet[:, :, coff:QC], sct[:, :, coff:QC], EXP)
                        if dg >= 0:
                            nc.gpsimd.tensor_mul(
                                tet[:, :, coff:coff + 128],
                                tet[:, :, coff:coff + 128],
                                mask_b(128))
                        tes[hp].append(tet)

            def ctx_phase(qc, op_inline=False):
                tes = all_tes[qc]
                for sti in range(4):
                    st = 4 * qc + sti
                    cxt = cxp.tile([128, 512], F32, tag="cx",
                                   name=f"cx{qc}_{sti}")
                    first = None
                    for hp in range(2):
                        for h2 in range(2):
                            h = 2 * hp + h2
                            for kt in range(st + 1):
                                m = nc.tensor.matmul(
                                    cxt[:, h * 128:h * 128 + 65],
                                    tes[hp][kt][:, h2,
                                                sti * 128:(sti + 1) * 128],
                                    tv[:, kt, h, :],
                                    start=(first is None), stop=False,
                                    skip_group_check=True)
                                if first is None:
                                    first = m
                                elif kt == 0:
                                    order(m, first)
                    rc = rcp.tile([128, 4], F32, tag="rc",
                                  name=f"rc{qc}_{sti}")
                    nc.vector.reciprocal(rc[:], apx(cxt, 64, [[128, 4], [1, 1]]))
                    tcn = tcnp.tile([128, 4, 64], BF16, tag="cn",
                                    name=f"cn{qc}_{sti}")
                    nc.vector.tensor_mul(
                        tcn[:], apx(cxt, 0, [[128, 4], [1, 64]]),
                        colb2(rc[:], 64))
                    for hp in range(2):
                        nc.sync.dma_start_transpose(
                            tct[:, hp, st * 128:(st + 1) * 128],
                            tcn[:, 2 * hp:2 * hp + 2, :])
                    if op_inline:
                        out_proj_sti(qc, sti)

            def out_proj_sti(qc, sti, act_copy=False):
                    st = 4 * qc + sti
                    for nn in range(2):
                        pyt = prp.tile([128, QC], F32, tag="pr",
                                       name=f"py{qc}_{sti}_{nn}")
                        for hp in range(2):
                            nc.tensor.matmul(
                                pyt[:],
                                tct[:, hp, st * 128:(st + 1) * 128],
                                two[:, hp, nn * QC:(nn + 1) * QC],
                                start=(hp == 0), stop=(hp == 1))
                        ty = typ.tile([128, QC], BF16, tag="y",
                                      name=f"y{qc}_{sti}_{nn}")
                        if act_copy:
                            nc.scalar.copy(ty[:], pyt[:])
                        else:
                            nc.vector.tensor_copy(ty[:], pyt[:])
                        nc.sync.dma_start(
                            y[st * 128:(st + 1) * 128,
                              nn * QC:(nn + 1) * QC], ty[:])

            def out_proj(qc, act_copy=False):
                for sti in range(4):
                    out_proj_sti(qc, sti, act_copy)

            # Emission = scheduler priority. The exp stream is the global
            # pacer: scores/exp chains first (chunk order) with the next
            # wave's q/k projection between them, then all deferrable work.
            CORDER = cfg.get("corder", [0, 1, 3, 2])
            # waves needed before a chunk's scores: all w <= qc
            emitted_qk = set()

            def need_qk(qc):
                for w_ in range(qc + 1):
                    if w_ not in emitted_qk:
                        emitted_qk.add(w_)
                        for r in range(2):
                            qk_unit(w_, r, "wq")
                            qk_unit(w_, r, "wk")

            need_qk(CORDER[0])
            for i, qc in enumerate(CORDER):
                scores_phase(qc)
                if i + 1 < NQC:
                    need_qk(CORDER[i + 1])
            emitted_v = set()

            def need_v(qc):
                for w_ in range(qc + 1):
                    if w_ not in emitted_v:
                        emitted_v.add(w_)
                        for p in range(2):
                            v_unit(w_, p)

            need_v(CORDER[0])
            for i, qc in enumerate(CORDER):
                if i + 1 < NQC:
                    need_v(CORDER[i + 1])
                ctx_phase(qc)
                out_proj(qc, act_copy=(i == NQC - 1))

    nc.compile()
    return nc


def _get_nc():
    if "nc" not in _CACHE:
        _CACHE["nc"] = _build_nc()
    return _CACHE["nc"]


def make_mask():
    kl = np.arange(128)[:, None]
    ql = np.arange(128)[None, :]
    return (ql >= kl).astype(BFNP)


def _fold(t):
    # [E, N] -> [128, NKS, 2, N] with e = ks*256 + sl*128 + p
    n = t.shape[1]
    return np.ascontiguousarray(
        t.reshape(NKS, 2, 128, n).transpose(2, 0, 1, 3))


def _q8(t):
    t8 = t.astype(E4NP)
    return t8, (t - t8.astype(np.float32)).astype(E4NP)


def shard_inputs(x, Wq, bq, Wk, bk, Wv, bv, Wo, bo):
    x = np.asarray(x, dtype=np.float32)
    scale = np.float32(1.0 / np.sqrt(D))
    mask = make_mask()
    ones = np.ones((1, 64), BFNP)
    in_maps = []
    xf = []
    for b in range(B):
        x8, rx8 = _q8(_fold(np.ascontiguousarray(x[b].T)))
        xf.append((x8, rx8))
    for c in range(NCORES):
        b, g = divmod(c, 4)
        cs = slice(g * EC, (g + 1) * EC)
        wq, rwq = _q8(_fold(np.asarray(Wq[:, cs]) * (scale * WSQ)))
        wk, rwk = _q8(_fold(np.asarray(Wk[:, cs]) * WSK))
        wv, rwv = _q8(_fold(np.asarray(Wv[:, cs]) * WSK))
        in_maps.append({
            "x8": xf[b][0], "rx8": xf[b][1],
            "wq8": wq, "rwq8": rwq,
            "wk8": wk, "rwk8": rwk,
            "wv8": wv, "rwv8": rwv,
            "wo": np.ascontiguousarray(
                np.asarray(Wo[cs, :]).reshape(2, 128, E).transpose(1, 0, 2)
            ).astype(BFNP),
            "bq": (np.asarray(bq[cs]) * scale).reshape(2, 128, 1).astype(np.float32),
            "bk": np.asarray(bk[cs]).reshape(2, 128, 1).astype(np.float32),
            "bv": np.asarray(bv[cs]).reshape(1, EC).astype(np.float32),
            "msk": mask,
            "ones": ones,
        })
    return in_maps


def combine_outputs(results, bo):
    y = np.zeros((B, S, E), np.float32)
    for c in range(NCORES):
        b = c // 4
        y[b] += np.asarray(results[c]["y"], dtype=np.float32)
    y += np.asarray(bo, dtype=np.float32)[None, None, :]
    return y


def kernel(x, Wq, bq, Wk, bk, Wv, bv, Wo, bo):
    from concourse.bass_utils import run_bass_kernel_spmd

    nc = _get_nc()
    in_maps = shard_inputs(x, Wq, bq, Wk, bk, Wv, bv, Wo, bo)
    try:
        res = run_bass_kernel_spmd(nc, in_maps, core_ids=list(range(NCORES)))
    except Exception:
        # transient device errors (e.g. a wedged core) usually clear on retry
        res = run_bass_kernel_spmd(nc, in_maps, core_ids=list(range(NCORES)))
    return combine_outputs(res.results, bo)
